# revision 1
# baseline (speedup 1.0000x reference)
"""AAEncoder (GNN message passing) on 8 NeuronCores.

Data-parallel over the hub-node axis N=1024: each of the 8 cores computes
128 rows of the N x N neighbor interaction + attention. The small parameter
set, the full neighbor position table (N*2 floats) and per-step velocities
are replicated on every device, so no cross-device communication is needed;
outputs are concatenated on the host.
"""
import numpy as np
import jax
import jax.numpy as jnp

N = 1024
D = 64
H = 8
DH = D // H
EPS = 1e-5
RADIUS = 50.0
M_DEV = 8
SHARD = N // M_DEV  # 128


def _ln(x, g, b):
    m = x.mean(-1, keepdims=True)
    v = ((x - m) ** 2).mean(-1, keepdims=True)
    return (x - m) / jnp.sqrt(v + EPS) * g + b


def _shard_forward(i0, pos_t, dpos, pad_t, p):
    """Compute output rows [i0, i0+SHARD) of the encoder.

    pos_t, dpos: [N,2] replicated; pad_t: [N] bool replicated.
    """
    f32 = jnp.float32
    pos_i = jax.lax.dynamic_slice(pos_t, (i0, 0), (SHARD, 2))   # [S,2]
    dpos_i = jax.lax.dynamic_slice(dpos, (i0, 0), (SHARD, 2))   # [S,2]

    ang = jnp.arctan2(dpos_i[:, 1], dpos_i[:, 0])
    c, s = jnp.cos(ang), jnp.sin(ang)
    R = jnp.stack([jnp.stack([c, -s], -1), jnp.stack([s, c], -1)], -2)  # [S,2,2]

    rel = pos_t[None, :, :] - pos_i[:, None, :]                 # [S,N,2]
    dist = jnp.linalg.norm(rel, axis=-1)                        # [S,N]
    col = jnp.arange(N)[None, :]
    row = i0 + jnp.arange(SHARD)[:, None]
    mask = (dist <= RADIUS) & (~pad_t)[None, :]
    mask = mask | (col == row)                                  # eye block
    rel_rot = jnp.einsum('ijx,ixy->ijy', rel, R)                # [S,N,2]
    dpos_rot = jnp.einsum('ix,ixy->iy', dpos_i, R)              # [S,2]

    # center embedding + norm1
    h = jax.nn.relu(_ln(dpos_rot @ p['ce_W1'].T + p['ce_b1'], p['ce_g1'], p['ce_be1']))
    h = jax.nn.relu(_ln(h @ p['ce_W2'].T + p['ce_b2'], p['ce_g2'], p['ce_be2']))
    center = _ln(h @ p['ce_W3'].T + p['ce_b3'], p['ce_g3'], p['ce_be3'])
    center = _ln(center, p['n1_g'], p['n1_b'])                  # [S,D]

    # neighbor embedding. e1 depends only on the source node j -> compute
    # once per j ([N,D]) and broadcast over the hub axis instead of [S,N,D].
    e0 = jax.nn.relu(_ln(rel_rot @ p['ne0_W1'].T + p['ne0_b1'],
                         p['ne0_g1'], p['ne0_be1'])) @ p['ne0_W2'].T + p['ne0_b2']
    e1 = jax.nn.relu(_ln(dpos @ p['ne1_W1'].T + p['ne1_b1'],
                         p['ne1_g1'], p['ne1_be1'])) @ p['ne1_W2'].T + p['ne1_b2']
    nbr = _ln(jax.nn.relu(_ln(e0 + e1[None, :, :], p['na_g1'], p['na_be1']))
              @ p['na_W'].T + p['na_b'], p['na_g2'], p['na_be2'])  # [S,N,D]

    # single-query masked multihead attention per hub node
    q = (center @ p['Wq'].T).reshape(SHARD, H, DH)
    k = (nbr @ p['Wk'].T).reshape(SHARD, N, H, DH)
    v = (nbr @ p['Wv'].T).reshape(SHARD, N, H, DH)
    logits = jnp.einsum('ihd,ijhd->ihj', q, k) / jnp.sqrt(jnp.asarray(DH, f32))
    logits = jnp.where(mask[:, None, :], logits, jnp.finfo(f32).min)
    attn = jax.nn.softmax(logits, axis=-1)                      # [S,H,N]
    mha = jnp.einsum('ihj,ijhd->ihd', attn, v).reshape(SHARD, D) @ p['Wo'].T

    gate = jax.nn.sigmoid(mha @ p['ih_W'].T + p['ih_b'] + center @ p['hh_W'].T + p['hh_b'])
    out = mha + gate * ((center @ p['self_W'].T + p['self_b']) - mha)
    out = _ln(out, p['n2_g'], p['n2_b'])
    ff = jax.nn.relu(out @ p['mlp_W1'].T + p['mlp_b1']) @ p['mlp_W2'].T + p['mlp_b2']
    return out + ff


_PMAPPED = None


def _get_pmapped():
    global _PMAPPED
    if _PMAPPED is None:
        _PMAPPED = jax.pmap(
            _shard_forward,
            in_axes=(0, None, None, None, None),
            static_broadcasted_argnums=(),
        )
    return _PMAPPED


def kernel(positions, bos_mask, padding_mask, t, params):
    del bos_mask  # unused by the math
    t = int(t)
    positions = np.asarray(positions, dtype=np.float32)
    pos_t = positions[:, t]                       # [N,2]
    dpos = positions[:, t] - positions[:, t - 1]  # [N,2]
    pad_t = np.asarray(padding_mask)[:, t]        # [N] bool
    p = {k: jnp.asarray(np.asarray(v)) for k, v in params.items()}

    i0s = jnp.arange(M_DEV, dtype=jnp.int32) * SHARD
    out = _get_pmapped()(i0s, jnp.asarray(pos_t), jnp.asarray(dpos),
                         jnp.asarray(pad_t), p)
    return np.asarray(out).reshape(N, D).astype(np.float32)
